# revision 7
# baseline (speedup 1.0000x reference)
"""GPRGNN Trainium2 kernel v2: scatter-free hop via tensor-engine segment sums.

Main path per hop: dest-sorted per (src window w, dest slot s) groups of 256
tokens (2 tiles x 128). Gather fp32 rows; build 0/1 indicator matrices
ind[tok, dest_in_slot] on DVE/ACT via is_equal(iota, rd); two accumulating
matmuls per group produce PSUM[128 dests, 64] = segment sums; one vector add
folds 4 groups into the SBUF state u. Tokens beyond 256 per group (~5k of
200k) go through the old dma_scatter_add path into a DRAM accumulator that
doubles as the AllGather input buffer (pre-filled with u_k = self-loop term).
AllGather outputs use Shared addr space; DMA gathers spread over 4 SWDGE
queues.
"""

import sys

sys.path.insert(0, "/root/problem")
sys.path.insert(0, "/opt/trn_rl_repo")

import numpy as np

import concourse.bacc as bacc
import concourse.bass as bass
import concourse.mybir as mybir
import concourse.tile as tile
from concourse.bass_utils import run_bass_kernel_spmd

N = 100000
E = 1600000
FIN = 512
HID = 256
C = 64
K = 10
NCORES = 8
SH = 12500
SHP = 12544          # 98 * 128
SLOTS = SHP // 128   # 98
NT = NCORES * SHP
TRASH = SHP
ACC_ROWS = SHP + 4096
CH = 1024
GRP = 256                    # tokens per (w, slot) group
NTOKW = SLOTS * GRP          # 25088 per window
NTOK = NCORES * NTOKW        # 200704 main tokens per core
NTILE = NCORES * SLOTS * 2   # 1568 tiles of 128
F32 = mybir.dt.float32
BF16 = mybir.dt.bfloat16
I16 = mybir.dt.int16

_cache = {}


def _wrap16(a):
    """[gt] -> [128, gt/16]: token j at [j%16 (+16m), j//16]."""
    return np.tile(a.reshape(-1, 16).T, (NCORES, 1)).copy()


def _plan_scatter(row, col):
    """Collision-free scatter plan for an edge subset (baseline algorithm).

    Returns (gw, sw, plan, gt): per-core wrapped idx arrays and per-window
    chunks (toff, tlen, [(soff, slen)...]), identical across cores.
    """
    Eo = len(row)
    if Eo == 0:
        return ([np.zeros((128, 0), np.int16)] * NCORES,
                [np.zeros((128, 0), np.int16)] * NCORES,
                [[] for _ in range(NCORES)], 0)
    dcore = col // SH
    wsrc = row // SH
    o1 = np.lexsort((col, wsrc, dcore))
    c1, w1, d1 = col[o1], wsrc[o1], dcore[o1]
    key = (d1 * NCORES + w1) * N + c1
    newrun = np.empty(Eo, bool)
    newrun[0] = True
    newrun[1:] = key[1:] != key[:-1]
    run_id = np.cumsum(newrun) - 1
    run_start = np.full(run_id[-1] + 1, Eo, np.int64)
    np.minimum.at(run_start, run_id, np.arange(Eo))
    rnd = np.arange(Eo) - run_start[run_id]
    o2 = np.lexsort((c1, rnd, w1, d1))
    rs = row[o1][o2]
    cs = c1[o2]
    ws_ = w1[o2]
    dc = d1[o2]
    rr = rnd[o2]
    gl = (rs % SH).astype(np.int16)
    sl = (cs % SH).astype(np.int16)

    rmax = int(rr.max()) + 1
    counts = np.bincount((dc * NCORES + ws_) * rmax + rr,
                         minlength=NCORES * NCORES * rmax
                         ).reshape(NCORES, NCORES, rmax)
    seg_end = np.cumsum(counts.reshape(-1))
    seg_start = (seg_end - counts.reshape(-1)).reshape(NCORES, NCORES, rmax)
    pseg = ((counts.max(axis=0) + 127) // 128) * 128

    plan = []
    seg_off = np.zeros((NCORES, rmax), np.int64)
    gt = 0
    for w in range(NCORES):
        chunks_w = []
        cur_off, cur_len, cur_scat = gt, 0, []
        for r in range(rmax):
            sl_len = int(pseg[w, r])
            if sl_len == 0:
                continue
            seg_off[w, r] = gt
            p = 0
            while p < sl_len:
                take = min(CH, sl_len - p, CH - cur_len)
                if take == 0:
                    chunks_w.append((cur_off, cur_len, cur_scat))
                    cur_off, cur_len, cur_scat = gt, 0, []
                    continue
                cur_scat.append((cur_len, take))
                cur_len += take
                p += take
                gt += take
                if cur_len == CH:
                    chunks_w.append((cur_off, cur_len, cur_scat))
                    cur_off, cur_len, cur_scat = gt, 0, []
        if cur_len:
            chunks_w.append((cur_off, cur_len, cur_scat))
        plan.append(chunks_w)
    if gt % 16:
        gt = ((gt + 15) // 16) * 16

    gidx = np.zeros((NCORES, gt), np.int16)
    sidx = np.empty((NCORES, gt), np.int16)
    sidx[:] = (TRASH + (np.arange(gt) % 4096)).astype(np.int16)
    for core in range(NCORES):
        for w in range(NCORES):
            for r in range(rmax):
                cnt = int(counts[core, w, r])
                if cnt == 0:
                    continue
                s0 = int(seg_start[core, w, r])
                o = int(seg_off[w, r])
                gidx[core, o:o + cnt] = gl[s0:s0 + cnt]
                sidx[core, o:o + cnt] = sl[s0:s0 + cnt]

    gw = [_wrap16(gidx[c]) for c in range(NCORES)]
    sw = [_wrap16(sidx[c]) for c in range(NCORES)]
    return gw, sw, plan, gt


def _preprocess2(edge_index):
    row = np.asarray(edge_index[0], dtype=np.int64)
    col = np.asarray(edge_index[1], dtype=np.int64)
    deg = (np.bincount(col, minlength=N) + 1.0).astype(np.float32)
    dinv = deg ** -0.5

    dcore = col // SH
    wsrc = row // SH
    gl = (row % SH).astype(np.int16)
    dl = col % SH
    slot = dl // 128
    rd = (dl % 128).astype(np.float32)

    gid = (dcore * NCORES + wsrc) * SLOTS + slot
    o = np.argsort(gid, kind="stable")
    gids = gid[o]
    ngroups = NCORES * NCORES * SLOTS
    first = np.full(ngroups, E, np.int64)
    np.minimum.at(first, gids, np.arange(E))
    rank = np.arange(E) - first[gids]

    sel = rank < GRP
    core_s = gids // (NCORES * SLOTS)
    pos = (gids % (NCORES * SLOTS)) * GRP + rank

    gmain = np.zeros((NCORES, NTOK), np.int16)
    rmain = np.full((NCORES, NTOK), -1.0, np.float32)
    gmain[core_s[sel], pos[sel]] = gl[o][sel]
    rmain[core_s[sel], pos[sel]] = rd[o][sel]

    gmw = [_wrap16(gmain[c]) for c in range(NCORES)]
    # rd matrix [128, NTILE]: token t*128+p -> [p, t]
    rdm = [np.ascontiguousarray(rmain[c].reshape(NTILE, 128).T)
           for c in range(NCORES)]

    ov = ~sel
    ogw, osw, oplan, ogt = _plan_scatter(row[o][ov], col[o][ov])

    iot = np.broadcast_to(np.arange(128, dtype=np.float32)[None, None, :],
                          (128, 8, 128)).copy()
    return dinv, deg, gmw, rdm, ogw, osw, oplan, ogt, iot


def _build2(oplan, ogt, temp_vals):
    nc = bacc.Bacc("TRN2", target_bir_lowering=False, debug=False,
                   num_devices=NCORES, num_swdge_queues=4)

    xT_h = nc.dram_tensor("xT", [FIN, SHP], F32, kind="ExternalInput").ap()
    w1_h = nc.dram_tensor("w1", [FIN, HID], F32, kind="ExternalInput").ap()
    w2_h = nc.dram_tensor("w2", [HID, C], F32, kind="ExternalInput").ap()
    b1_h = nc.dram_tensor("b1", [HID, 1], F32, kind="ExternalInput").ap()
    b2_h = nc.dram_tensor("b2b", [128, C], F32, kind="ExternalInput").ap()
    dv_h = nc.dram_tensor("dv", [128, SLOTS], F32, kind="ExternalInput").ap()
    d2_h = nc.dram_tensor("d2", [128, SLOTS], F32, kind="ExternalInput").ap()
    dvi_h = nc.dram_tensor("dvi", [128, SLOTS], F32, kind="ExternalInput").ap()
    gim_h = nc.dram_tensor("gim", [128, NTOK // 16], I16,
                           kind="ExternalInput").ap()
    rdm_h = nc.dram_tensor("rdm", [128, NTILE], F32, kind="ExternalInput").ap()
    iot_h = nc.dram_tensor("iot", [128, 8 * 128], F32,
                           kind="ExternalInput").ap()
    ogt16 = max(ogt // 16, 1)
    gov_h = nc.dram_tensor("gov", [128, ogt16], I16, kind="ExternalInput").ap()
    sov_h = nc.dram_tensor("sov", [128, ogt16], I16, kind="ExternalInput").ap()
    out_h = nc.dram_tensor("out", [SHP, C], F32, kind="ExternalOutput").ap()

    tabA_t = nc.dram_tensor("tabAsh", [NT, C], F32, kind="Internal",
                            addr_space="Shared")
    tabB_t = nc.dram_tensor("tabBsh", [NT, C], F32, kind="Internal",
                            addr_space="Shared")

    with tile.TileContext(nc, trace_sim=False) as tc:
        with (
            tc.tile_pool(name="persist", bufs=1) as pp,
            tc.tile_pool(name="dram", bufs=1, space="DRAM") as dp,
            tc.tile_pool(name="mlp", bufs=3) as mp,
            tc.tile_pool(name="psum", bufs=2, space="PSUM") as psp,
            tc.tile_pool(name="psum2", bufs=1, space="PSUM") as psp2,
            tc.tile_pool(name="gb", bufs=6) as gp,
            tc.tile_pool(name="ib", bufs=4) as ip,
            tc.tile_pool(name="psg", bufs=5, space="PSUM") as qp,
        ):
            u = pp.tile([128, SLOTS, C], F32)
            Hacc = pp.tile([128, SLOTS, C], F32)
            ebuf = pp.tile([128, SLOTS, C], F32)
            w1sb = pp.tile([128, 4, HID], F32)
            w2sb = pp.tile([128, 2, C], F32)
            b1sb = pp.tile([128, 2], F32)
            b2sb = pp.tile([128, C], F32)
            dv = pp.tile([128, SLOTS], F32)
            d2 = pp.tile([128, SLOTS], F32)
            dvi = pp.tile([128, SLOTS], F32)
            gim = pp.tile([128, NTOK // 16], I16)
            rdm = pp.tile([128, NTILE], F32)
            iot = pp.tile([128, 8, 128], F32)
            gov = pp.tile([128, ogt16], I16)
            sov = pp.tile([128, ogt16], I16)
            mx = pp.tile([128, SLOTS], F32)
            sm = pp.tile([128, SLOTS], F32)

            nc.sync.dma_start(w1sb[:], w1_h.rearrange("(k p) h -> p k h", p=128))
            nc.sync.dma_start(w2sb[:], w2_h.rearrange("(k p) f -> p k f", p=128))
            nc.sync.dma_start(b1sb[:], b1_h.rearrange("(k p) o -> p (k o)", p=128))
            nc.sync.dma_start(b2sb[:], b2_h)
            nc.sync.dma_start(dv[:], dv_h)
            nc.sync.dma_start(d2[:], d2_h)
            nc.sync.dma_start(dvi[:], dvi_h)
            nc.sync.dma_start(gim[:], gim_h)
            nc.sync.dma_start(rdm[:], rdm_h)
            nc.sync.dma_start(iot[:], iot_h.rearrange("p (t j) -> p t j", j=128))
            if ogt:
                nc.sync.dma_start(gov[:], gov_h)
                nc.sync.dma_start(sov[:], sov_h)

            tabA = tabA_t.ap()
            tabB = tabB_t.ap()
            agin = dp.tile([ACC_ROWS, C], F32)

            def wrapped(dram_ap):
                return dram_ap.rearrange("(c p) f -> p c f", p=128)

            # ---- MLP: u0 = dinv * (relu(x@W1+b1)@W2+b2) ----
            moff = 0
            slot = 0
            while moff < SHP:
                mw = min(512, SHP - moff)
                h1 = []
                for hb in range(2):
                    ps = psp.tile([128, 512], F32, tag="ps")
                    for kk in range(4):
                        xt = mp.tile([128, 512], F32, tag="xt")
                        nc.sync.dma_start(xt[:, :mw],
                                          xT_h[kk * 128:(kk + 1) * 128,
                                               moff:moff + mw])
                        nc.tensor.matmul(ps[:, :mw],
                                         lhsT=w1sb[:, kk, hb * 128:(hb + 1) * 128],
                                         rhs=xt[:, :mw],
                                         start=(kk == 0), stop=(kk == 3))
                    ht = mp.tile([128, 512], F32, tag="ht")
                    nc.scalar.activation(ht[:, :mw], ps[:, :mw],
                                         mybir.ActivationFunctionType.Relu,
                                         bias=b1sb[:, hb:hb + 1], scale=1.0)
                    h1.append(ht)
                for st in range(mw // 128):
                    ps2 = psp2.tile([128, C], F32, tag="ps2")
                    for hb in range(2):
                        nc.tensor.matmul(ps2[:],
                                         lhsT=h1[hb][:, st * 128:(st + 1) * 128],
                                         rhs=w2sb[:, hb, :],
                                         start=(hb == 0), stop=(hb == 1))
                    t1 = mp.tile([128, C], F32, tag="t1")
                    nc.vector.tensor_add(t1[:], ps2[:], b2sb[:])
                    nc.vector.tensor_scalar(u[:, slot, :], t1[:],
                                            dv[:, slot:slot + 1], None,
                                            mybir.AluOpType.mult)
                    slot += 1
                moff += mw

            nc.vector.tensor_scalar(Hacc[:], u[:], float(temp_vals[0]), None,
                                    mybir.AluOpType.mult)

            tabs = [tabA, tabB]
            qc = 0
            for k in range(K):
                tab = tabs[k % 2]
                # broadcast u_k; agin also becomes the overflow accumulator
                nc.sync.dma_start(wrapped(agin[:SHP, :]), u[:])
                nc.gpsimd.collective_compute(
                    "AllGather", mybir.AluOpType.bypass,
                    replica_groups=[list(range(NCORES))],
                    ins=[agin[:SHP, :].opt()], outs=[tab.opt()])

                nc.vector.memset(u[:], 0.0)

                # overflow tokens via scatter-add into agin (holds u_k).
                # Emitted FIRST so the serial scatter chain (WAW on agin)
                # runs concurrently with the main gather/matmul pipeline
                # instead of as an end-of-hop tail.
                for w in range(NCORES):
                    srcw = tab[w * SHP:(w + 1) * SHP, :]
                    for (toff, tlen, scats) in oplan[w]:
                        og = gp.tile([128, 8, C], F32, tag="og")
                        nc.gpsimd.dma_gather(
                            og[:, :tlen // 128, :], srcw,
                            gov[:, toff // 16:(toff + tlen) // 16],
                            tlen, tlen, C, queue_num=qc % 4)
                        qc += 1
                        for (soff, slen) in scats:
                            nc.gpsimd.dma_scatter_add(
                                agin[:, :],
                                og[:, soff // 128:(soff + slen) // 128, :],
                                sov[:, (toff + soff) // 16:
                                    (toff + soff + slen) // 16],
                                slen, slen, C, queue_num=qc % 4)
                            qc += 1

                for w in range(NCORES):
                    srcw = tab[w * SHP:(w + 1) * SHP, :]
                    woff = w * NTOKW
                    coff = 0
                    while coff < NTOKW:
                        ctok = min(CH, NTOKW - coff)
                        nslot = ctok // 128
                        ngr = ctok // GRP
                        toff = woff + coff
                        g = gp.tile([128, 8, C], F32, tag="g")
                        nc.gpsimd.dma_gather(
                            g[:, :nslot, :], srcw,
                            gim[:, toff // 16:(toff + ctok) // 16],
                            ctok, ctok, C, queue_num=qc % 4)
                        qc += 1
                        # bf16 operands enable Fast Weight Load on PE
                        gb = gp.tile([128, 8, C], BF16, tag="gb")
                        nc.scalar.activation(
                            gb[:, :nslot, :], g[:, :nslot, :],
                            mybir.ActivationFunctionType.Copy)
                        ind = ip.tile([128, 8, 128], BF16, tag="ind")
                        tt0 = toff // 128
                        nc.vector.tensor_tensor(
                            ind[:, :nslot, :],
                            rdm[:, tt0:tt0 + nslot, None].to_broadcast(
                                [128, nslot, 128]),
                            iot[:, :nslot, :],
                            mybir.AluOpType.is_equal)
                        ps4 = qp.tile([128, 4, C], F32, tag="ps4")
                        for gq in range(ngr):
                            for j in range(2):
                                t = 2 * gq + j
                                nc.tensor.matmul(ps4[:, gq, :],
                                                 lhsT=ind[:, t, :],
                                                 rhs=gb[:, t, :],
                                                 start=(j == 0), stop=(j == 1))
                        s0 = coff // GRP
                        nc.vector.tensor_tensor(u[:, s0:s0 + ngr, :],
                                                u[:, s0:s0 + ngr, :],
                                                ps4[:, :ngr, :],
                                                mybir.AluOpType.add)
                        coff += ctok

                # u_{k+1} = d2 * (u_k + ovf + matmul messages)
                nc.sync.dma_start(ebuf[:], wrapped(agin[:SHP, :]))
                nc.vector.tensor_tensor(u[:], u[:], ebuf[:],
                                        mybir.AluOpType.add)
                nc.vector.tensor_tensor(
                    u[:], u[:],
                    d2[:, :, None].to_broadcast([128, SLOTS, C]),
                    mybir.AluOpType.mult)
                nc.vector.scalar_tensor_tensor(
                    Hacc[:], u[:], float(temp_vals[k + 1]), Hacc[:],
                    mybir.AluOpType.mult, mybir.AluOpType.add)

            # ---- hidden = Hacc * sqrt(deg) ; log_softmax ----
            nc.vector.tensor_tensor(
                ebuf[:], Hacc[:],
                dvi[:, :, None].to_broadcast([128, SLOTS, C]),
                mybir.AluOpType.mult)
            nc.vector.tensor_reduce(mx[:], ebuf[:], mybir.AxisListType.X,
                                    mybir.AluOpType.max)
            nc.vector.tensor_tensor(
                ebuf[:], ebuf[:],
                mx[:, :, None].to_broadcast([128, SLOTS, C]),
                mybir.AluOpType.subtract)
            s0 = 0
            while s0 < SLOTS:
                sn = min(8, SLOTS - s0)
                ex = gp.tile([128, 8, C], F32, tag="g")
                nc.scalar.activation(ex[:, :sn, :], ebuf[:, s0:s0 + sn, :],
                                     mybir.ActivationFunctionType.Exp)
                nc.vector.tensor_reduce(sm[:, s0:s0 + sn], ex[:, :sn, :],
                                        mybir.AxisListType.X,
                                        mybir.AluOpType.add)
                s0 += sn
            nc.scalar.activation(sm[:], sm[:],
                                 mybir.ActivationFunctionType.Ln)
            nc.vector.tensor_tensor(
                ebuf[:], ebuf[:],
                sm[:, :, None].to_broadcast([128, SLOTS, C]),
                mybir.AluOpType.subtract)
            nc.sync.dma_start(wrapped(out_h), ebuf[:])

    nc.compile()
    return nc


def _make_in_maps(inputs, pre):
    dinv, deg, gmw, rdm, ogw, osw, oplan, ogt, iot = pre
    x = np.asarray(inputs["x"], dtype=np.float32)
    W1 = np.asarray(inputs["W1"], dtype=np.float32)
    b1 = np.asarray(inputs["b1"], dtype=np.float32)
    W2 = np.asarray(inputs["W2"], dtype=np.float32)
    b2 = np.asarray(inputs["b2"], dtype=np.float32)
    b2b = np.broadcast_to(b2[None, :], (128, C)).copy()
    iot_flat = np.ascontiguousarray(iot.reshape(128, 8 * 128))
    ogt16 = max(ogt // 16, 1)
    in_maps = []
    for core in range(NCORES):
        lo = core * SH
        xs = np.zeros((SHP, FIN), np.float32)
        xs[:SH] = x[lo:lo + SH]
        dloc = np.zeros(SHP, np.float32)
        dloc[:SH] = dinv[lo:lo + SH]
        d2loc = np.zeros(SHP, np.float32)
        d2loc[:SH] = 1.0 / deg[lo:lo + SH]
        dviloc = np.zeros(SHP, np.float32)
        dviloc[:SH] = np.sqrt(deg[lo:lo + SH])

        def wrapv(v):
            return v.reshape(SLOTS, 128).T.copy()

        gv = ogw[core] if ogt else np.zeros((128, ogt16), np.int16)
        sv = osw[core] if ogt else np.zeros((128, ogt16), np.int16)
        in_maps.append({
            "xT": np.ascontiguousarray(xs.T),
            "w1": W1, "w2": W2,
            "b1": b1[:, None].copy(), "b2b": b2b,
            "dv": wrapv(dloc), "d2": wrapv(d2loc), "dvi": wrapv(dviloc),
            "gim": gmw[core], "rdm": rdm[core], "iot": iot_flat,
            "gov": gv, "sov": sv,
        })
    return in_maps


_pre_cache = {}


def kernel(**inputs):
    temp = np.asarray(inputs["temp"], dtype=np.float32)
    ei = np.asarray(inputs["edge_index"])
    pkey = (ei.shape, ei[:, ::4097].tobytes())
    if pkey not in _pre_cache:
        _pre_cache[pkey] = _preprocess2(ei)
    pre = _pre_cache[pkey]
    oplan, ogt = pre[6], pre[7]

    key = (ogt,
           tuple(tuple((o, l, tuple(s)) for (o, l, s) in cw) for cw in oplan),
           tuple(np.round(temp, 10)))
    if key not in _cache:
        _cache[key] = _build2(oplan, ogt, [float(t) for t in temp])
    nc = _cache[key]

    in_maps = _make_in_maps(inputs, pre)
    res = run_bass_kernel_spmd(nc, in_maps, list(range(NCORES)))
    outs = [res.results[c]["out"] for c in range(NCORES)]
    return np.concatenate([o[:SH] for o in outs], axis=0)
